# revision 16
# baseline (speedup 1.0000x reference)
"""Bass/Trainium2 kernel for the 2-layer GAT problem (nn_GAT_79998060855611).

Strategy: row-shard the N x N attention across 8 NeuronCores (each core owns
N/8 = 512 query nodes).  Scores live in transposed layout [m, n] so the
softmax denominator falls out of the value-aggregation matmul via a ones
column in its lhsT.  The whole score computation
    V[m, n] = lrelu(s_src[n] + s_tgt[m] + mask[n, m])
is ONE custom DVE op per tile: Src0 = s_src broadcast rows (f32), Src1 =
transposed mask (bf16), C1 = per-partition s_tgt column (f32).  exp is a
single ScalarE pass.  All matmuls are bf16 with hi/lo-split accumulation.
The layers exchange the tiny layer-1 projections via an on-chip AllGather.
The final normalisation (divide by the attention row-sums) and the output
transpose are done on the host ([65, 512] per core).
"""
import os
import numpy as np
import ml_dtypes

import concourse.bass as bass
import concourse.tile as tile
import concourse.dve_ops as dve_ops
from concourse import bacc, mybir
from concourse.bass_utils import run_bass_kernel_spmd
from concourse.dve_spec import Spec, Src0, Src1, C0, C1, maxx, lower
from concourse.dve_uop import DveOpSpec

bf16 = ml_dtypes.bfloat16
F32 = mybir.dt.float32
BF16 = mybir.dt.bfloat16
F16 = mybir.dt.float16
AF = mybir.ActivationFunctionType

N = 4096
FIN = 128
H0, F0 = 8, 64
OUT0 = H0 * F0          # 512
F1 = 64
NCORES = 8
NL = N // NCORES        # 512 queries per core
NEG = 0.2
NMC = N // 128          # 32 m-chunks


# ---------------------------------------------------------------- custom op
def _register_score_lrelu():
    """out = lrelu(Src0 + Src1 + C1) with slope C0."""
    name = "GAT_SCORE_LRELU"
    for op in dve_ops.OPS:
        if op.name == name:
            return op
    u = Src0 + Src1 + C1
    spec = Spec(body=maxx(u * C0, u))
    opcode = dve_ops._CUSTOM_DVE_ROW_BASE + len(dve_ops.OPS)
    shas = {}
    for ver in ("v3", "v4"):
        s = DveOpSpec(name=name, opcode=opcode, uops=lower(spec, ver=ver), rd1_en=True)
        shas[ver] = s.sha(ver)
    op = dve_ops.DveOp(name, spec, subdim=False, uops_sha=shas)
    dve_ops.OPS.append(op)
    dve_ops._SUB_OPCODE_FOR_NAME[name] = opcode
    dve_ops.CUSTOM_DVE_SPECS[name] = spec
    return op


SCORE_LRELU = _register_score_lrelu()


def _hilo(a):
    hi = a.astype(bf16)
    lo = (a - hi.astype(np.float32)).astype(bf16)
    return hi, lo


def _bcast_rows(d_handle, row, width, parts=128):
    """DRAM AP reading one row of a [rows, width] f32 tensor, replicated
    across `parts` partitions."""
    return bass.AP(tensor=d_handle, offset=row * width,
                   ap=[[0, parts], [1, width]])


# ---------------------------------------------------------------- program
def build_program(sim_mode=False):
    nc = bacc.Bacc("TRN2", target_bir_lowering=False, debug=False,
                   num_devices=NCORES)

    def din(name, shape, dt=BF16):
        return nc.dram_tensor(name, shape, dt, kind="ExternalInput")

    d_xT_hi = din("xT_hi", [FIN, N]); d_xT_lo = din("xT_lo", [FIN, N])
    d_xTl_hi = din("xTl_hi", [FIN, NL]); d_xTl_lo = din("xTl_lo", [FIN, NL])
    d_W0_hi = din("W0_hi", [FIN, OUT0]); d_W0_lo = din("W0_lo", [FIN, OUT0])
    d_WA0_hi = din("WA0_hi", [FIN, 2 * H0]); d_WA0_lo = din("WA0_lo", [FIN, 2 * H0])
    d_sb0 = din("sb0", [2 * H0, 1], F32)        # b0 @ A0, column
    d_sb0r = din("sb0r", [2 * H0], F32)         # b0 @ A0, row (broadcast source)
    d_b0r_hi = din("b0r_hi", [1, OUT0]); d_b0r_lo = din("b0r_lo", [1, OUT0])
    d_W1s_hi = din("W1s_hi", [F0, H0, F1]); d_W1s_lo = din("W1s_lo", [F0, H0, F1])
    d_WA1s_hi = din("WA1s_hi", [F0, H0, 2]); d_WA1s_lo = din("WA1s_lo", [F0, H0, 2])
    d_b1r_hi = din("b1r_hi", [1, F1]); d_b1r_lo = din("b1r_lo", [1, F1])
    d_sb1 = din("sb1", [2, 1], F32)
    d_mask = din("maskT", [N, NL])

    d_out = nc.dram_tensor("outT", [F1 + 1, NL], F32, kind="ExternalOutput")

    # internal DRAM: collective bounce + row-broadcast scratch
    d_cin = nc.dram_tensor("gat_cin", [NL, 68], BF16)
    d_cout = nc.dram_tensor("gat_cout", [N, 68], BF16, addr_space="Shared")
    d_srows = nc.dram_tensor("gat_srows", [2 * H0, NL], F16)
    d_srow1 = nc.dram_tensor("gat_srow1", [1, NL], F16)

    with tile.TileContext(nc) as tc:
        with (
            tc.tile_pool(name="const", bufs=1) as cp,
            tc.tile_pool(name="work", bufs=3) as wp,
            tc.tile_pool(name="psS", bufs=2, space="PSUM") as psS,
            tc.tile_pool(name="psAcc", bufs=1, space="PSUM") as psA,
        ):
            # ---------------- loads
            m_mask = cp.tile([128, NMC, NL], BF16)
            _mview = d_mask.ap().rearrange("(t p) n -> p t n", p=128)
            for _mq in range(8):
                nc.sync.dma_start(m_mask[:, _mq * 4:(_mq + 1) * 4, :],
                                  _mview[:, _mq * 4:(_mq + 1) * 4, :])

            t_xT_hi = cp.tile([FIN, N], BF16); nc.sync.dma_start(t_xT_hi, d_xT_hi[:, :])
            t_xT_lo = cp.tile([FIN, N], BF16); nc.sync.dma_start(t_xT_lo, d_xT_lo[:, :])
            t_xTl_hi = cp.tile([FIN, NL], BF16); nc.sync.dma_start(t_xTl_hi, d_xTl_hi[:, :])
            t_xTl_lo = cp.tile([FIN, NL], BF16); nc.sync.dma_start(t_xTl_lo, d_xTl_lo[:, :])
            t_W0_hi = cp.tile([FIN, OUT0], BF16); nc.sync.dma_start(t_W0_hi, d_W0_hi[:, :])
            t_W0_lo = cp.tile([FIN, OUT0], BF16); nc.sync.dma_start(t_W0_lo, d_W0_lo[:, :])
            t_WA0_hi = cp.tile([FIN, 2 * H0], BF16); nc.sync.dma_start(t_WA0_hi, d_WA0_hi[:, :])
            t_WA0_lo = cp.tile([FIN, 2 * H0], BF16); nc.sync.dma_start(t_WA0_lo, d_WA0_lo[:, :])
            t_sb0 = cp.tile([2 * H0, 1], F32); nc.sync.dma_start(t_sb0, d_sb0[:, :])
            t_sb0b = cp.tile([128, 2 * H0], F32)
            nc.sync.dma_start(t_sb0b, bass.AP(tensor=d_sb0r, offset=0,
                                              ap=[[0, 128], [1, 2 * H0]]))
            t_b0r_hi = cp.tile([1, OUT0], BF16); nc.sync.dma_start(t_b0r_hi, d_b0r_hi[:, :])
            t_b0r_lo = cp.tile([1, OUT0], BF16); nc.sync.dma_start(t_b0r_lo, d_b0r_lo[:, :])
            t_W1s_hi = cp.tile([F0, H0, F1], BF16); nc.sync.dma_start(t_W1s_hi, d_W1s_hi[:, :, :])
            t_W1s_lo = cp.tile([F0, H0, F1], BF16); nc.sync.dma_start(t_W1s_lo, d_W1s_lo[:, :, :])
            t_WA1s_hi = cp.tile([F0, H0, 2], BF16); nc.sync.dma_start(t_WA1s_hi, d_WA1s_hi[:, :, :])
            t_WA1s_lo = cp.tile([F0, H0, 2], BF16); nc.sync.dma_start(t_WA1s_lo, d_WA1s_lo[:, :, :])
            t_b1r_hi = cp.tile([1, F1], BF16); nc.sync.dma_start(t_b1r_hi, d_b1r_hi[:, :])
            t_b1r_lo = cp.tile([1, F1], BF16); nc.sync.dma_start(t_b1r_lo, d_b1r_lo[:, :])
            t_sb1 = cp.tile([2, 1], F32); nc.sync.dma_start(t_sb1, d_sb1[:, :])

            t_ones1 = cp.tile([1, 128], BF16)
            nc.vector.memset(t_ones1, 1.0)
            t_ones164 = cp.tile([1, F0], BF16)
            nc.vector.memset(t_ones164, 1.0)

            # ---------------- proj0_ext + s_all0_nat (s_tgt columns)
            proj0_ext = cp.tile([128, NMC, H0 * (F0 + 1)], BF16)
            p0v = proj0_ext.rearrange("p t (h f) -> p t h f", h=H0)
            nc.vector.memset(p0v[:, :, :, F0], 1.0)
            s_all0_nat = cp.tile([128, NMC, 2 * H0], F32)
            for mc in range(NMC):
                ps = psS.tile([128, 528], F32, tag="scratch")
                pp = ps[:, 0:OUT0]
                pq = ps[:, OUT0:OUT0 + 2 * H0]
                xs_hi = t_xT_hi[:, mc * 128:(mc + 1) * 128]
                xs_lo = t_xT_lo[:, mc * 128:(mc + 1) * 128]
                nc.tensor.matmul(pp, lhsT=xs_hi, rhs=t_W0_hi, start=True, stop=False)
                nc.tensor.matmul(pp, lhsT=xs_hi, rhs=t_W0_lo, start=False, stop=False)
                nc.tensor.matmul(pp, lhsT=xs_lo, rhs=t_W0_hi, start=False, stop=False)
                nc.tensor.matmul(pp, lhsT=t_ones1, rhs=t_b0r_hi, start=False, stop=False)
                nc.tensor.matmul(pp, lhsT=t_ones1, rhs=t_b0r_lo, start=False, stop=True)
                nc.tensor.matmul(pq, lhsT=xs_hi, rhs=t_WA0_hi, start=True, stop=False)
                nc.tensor.matmul(pq, lhsT=xs_hi, rhs=t_WA0_lo, start=False, stop=False)
                nc.tensor.matmul(pq, lhsT=xs_lo, rhs=t_WA0_hi, start=False, stop=True)
                nc.scalar.copy(p0v[:, mc, :, 0:F0],
                               pp.rearrange("p (h f) -> p h f", h=H0))
                nc.vector.tensor_add(s_all0_nat[:, mc, :], pq, t_sb0b)

            # ---------------- s_src rows (local, T layout), broadcast via DRAM
            s_l0 = cp.tile([2 * H0, NL], F32)
            ps = psS.tile([128, 528], F32, tag="scratch")
            pp = ps[0:2 * H0, 0:NL]
            nc.tensor.matmul(pp, lhsT=t_WA0_hi, rhs=t_xTl_hi, start=True, stop=False)
            nc.tensor.matmul(pp, lhsT=t_WA0_lo, rhs=t_xTl_hi, start=False, stop=False)
            nc.tensor.matmul(pp, lhsT=t_WA0_hi, rhs=t_xTl_lo, start=False, stop=True)
            nc.scalar.activation(s_l0, pp, AF.Identity, bias=t_sb0)
            s_l0h = cp.tile([2 * H0, NL], F16)
            nc.vector.tensor_copy(s_l0h, s_l0)
            nc.sync.dma_start(d_srows[:, :], s_l0h)
            USrc = cp.tile([128, H0, NL], F16)
            for h in range(H0):
                nc.sync.dma_start(USrc[:, h, :], _bcast_rows(d_srows, h, NL))

            # ---------------- layer-0 main loop
            hT_hi = [cp.tile([F0, NL], BF16, name=f"hthi{h}", tag=f"hthi{h}")
                     for h in range(H0)]
            hT_lo = [cp.tile([F0, NL], BF16, name=f"htlo{h}", tag=f"htlo{h}")
                     for h in range(H0)]
            rec_rows = cp.tile([1, 2, H0, NL], BF16)

            for g in range(2):
                accs = [psA.tile([F0 + 1, NL], F32, name=f"acc{hh}", tag=f"acc{hh}")
                        for hh in range(4)]
                for mcp in range(NMC // 4):
                    for hh in range(4):
                        h = 4 * g + hh
                        tV = wp.tile([128, 2048], F32, tag="V", bufs=2)
                        tP = wp.tile([128, 2048], BF16, tag="P", bufs=2)
                        for sub in range(4):
                            mc = 4 * mcp + sub
                            nc.vector._custom_dve(
                                SCORE_LRELU,
                                out=tV[:, sub * 512:(sub + 1) * 512],
                                in0=USrc[:, h, :],
                                in1=m_mask[:, mc, :],
                                s0=NEG,
                                s1=s_all0_nat[:, mc, H0 + h:H0 + h + 1])
                        nc.scalar.activation(tP, tV, AF.Exp)
                        for sub in range(4):
                            mc = 4 * mcp + sub
                            nc.tensor.matmul(
                                accs[hh],
                                lhsT=p0v[:, mc, h, :],
                                rhs=tP[:, sub * 512:(sub + 1) * 512],
                                start=(mc == 0), stop=(mc == NMC - 1),
                                skip_group_check=True)
                # normalize + split h_T for this group
                sums0g = wp.tile([4, NL], F32, tag="sums0", bufs=2)
                for hh in range(4):
                    sums_stage = wp.tile([F0 + 1, NL], F32, tag="sums_stage", bufs=2)
                    nc.scalar.copy(sums_stage[F0:F0 + 1, :], accs[hh][F0:F0 + 1, :])
                    nc.sync.dma_start(sums0g[hh:hh + 1, :], sums_stage[F0:F0 + 1, :])
                rec = wp.tile([4, NL], F32, tag="rec", bufs=2)
                nc.vector.reciprocal(rec, sums0g)
                rec_hi = wp.tile([4, NL], BF16, tag="rechi", bufs=2)
                nc.vector.tensor_copy(rec_hi, rec)
                rec_hif = wp.tile([4, NL], F32, tag="rechif", bufs=2)
                nc.vector.tensor_copy(rec_hif, rec_hi)
                rec_lo = wp.tile([4, NL], BF16, tag="reclo", bufs=2)
                nc.vector.tensor_sub(rec_lo, rec, rec_hif)
                for hh in range(4):
                    h = 4 * g + hh
                    nc.sync.dma_start(rec_rows[0:1, 0, h, :], rec_hi[hh:hh + 1, :])
                    nc.sync.dma_start(rec_rows[0:1, 1, h, :], rec_lo[hh:hh + 1, :])
                for hh in range(4):
                    h = 4 * g + hh
                    psB = psS.tile([128, 528], F32, tag="scratch")
                    pb = psB[0:F0, 0:NL]
                    nc.tensor.matmul(pb, lhsT=t_ones164, rhs=rec_rows[0:1, 0, h, :],
                                     start=True, stop=False)
                    nc.tensor.matmul(pb, lhsT=t_ones164, rhs=rec_rows[0:1, 1, h, :],
                                     start=False, stop=True)
                    tb = wp.tile([F0, NL], F32, tag="tb", bufs=2)
                    nc.scalar.copy(tb, pb)
                    tHf = wp.tile([F0, NL], F32, tag="tHf", bufs=2)
                    nc.vector.tensor_mul(tHf, accs[hh][0:F0, :], tb)
                    nc.vector.tensor_copy(hT_hi[h], tHf)
                    tmp = wp.tile([F0, NL], F32, tag="tmp", bufs=2)
                    nc.vector.tensor_copy(tmp, hT_hi[h])
                    nc.vector.tensor_sub(hT_lo[h], tHf, tmp)

            # ---------------- proj1 local [NL, F1] f32 + s_all1_T local [2, NL]
            proj1n = cp.tile([128, 4, F1], BF16)
            for nc4 in range(4):
                ps = psS.tile([128, 528], F32, tag="scratch")
                pp = ps[:, 0:F1]
                for h in range(H0):
                    w_hi = t_W1s_hi[:, h, :]
                    w_lo = t_W1s_lo[:, h, :]
                    hh_ = hT_hi[h][:, nc4 * 128:(nc4 + 1) * 128]
                    hl_ = hT_lo[h][:, nc4 * 128:(nc4 + 1) * 128]
                    nc.tensor.matmul(pp, lhsT=hh_, rhs=w_hi, start=(h == 0), stop=False)
                    nc.tensor.matmul(pp, lhsT=hh_, rhs=w_lo, start=False, stop=False)
                    nc.tensor.matmul(pp, lhsT=hl_, rhs=w_hi, start=False, stop=False)
                nc.tensor.matmul(pp, lhsT=t_ones1, rhs=t_b1r_hi, start=False, stop=False)
                nc.tensor.matmul(pp, lhsT=t_ones1, rhs=t_b1r_lo, start=False, stop=True)
                nc.scalar.copy(proj1n[:, nc4, :], pp)
            ps1 = psS.tile([128, 528], F32, tag="scratch")
            pp1 = ps1[0:2, 0:NL]
            for h in range(H0):
                wa_hi = t_WA1s_hi[:, h, :]
                wa_lo = t_WA1s_lo[:, h, :]
                nc.tensor.matmul(pp1, lhsT=wa_hi, rhs=hT_hi[h], start=(h == 0), stop=False)
                nc.tensor.matmul(pp1, lhsT=wa_lo, rhs=hT_hi[h], start=False, stop=False)
                nc.tensor.matmul(pp1, lhsT=wa_hi, rhs=hT_lo[h], start=False,
                                 stop=(h == H0 - 1))
            s1l = cp.tile([2, NL], F32)
            nc.scalar.activation(s1l, pp1, AF.Identity, bias=t_sb1)
            s1l_hi = cp.tile([2, NL], BF16)
            nc.vector.tensor_copy(s1l_hi, s1l)
            s1l_hif = cp.tile([2, NL], F32)
            nc.vector.tensor_copy(s1l_hif, s1l_hi)
            s1l_lo = cp.tile([2, NL], BF16)
            nc.vector.tensor_sub(s1l_lo, s1l, s1l_hif)

            # ---------------- gather: [NL, 68] bf16 = proj1 | tgt_hi | tgt_lo | pad
            nc.sync.dma_start(
                d_cin.ap().rearrange("(c p) f -> p c f", p=128)[:, :, 0:F1], proj1n)
            nc.sync.dma_start(
                bass.AP(tensor=d_cin, offset=F1, ap=[[68, NL], [1, 1]]), s1l_hi[1:2, :])
            nc.sync.dma_start(
                bass.AP(tensor=d_cin, offset=F1 + 1, ap=[[68, NL], [1, 1]]), s1l_lo[1:2, :])
            if sim_mode:
                for c in range(NCORES):
                    nc.sync.dma_start(d_cout[c * NL:(c + 1) * NL, :], d_cin[:, :])
            else:
                nc.gpsimd.collective_compute(
                    "AllGather", mybir.AluOpType.bypass,
                    replica_groups=[list(range(NCORES))],
                    ins=[d_cin.ap().opt()], outs=[d_cout.ap().opt()])

            # proj1_ext [128, 32, 65] bf16 + s_tgt1_nat [128, 32] f32
            proj1_ext = cp.tile([128, NMC, F1 + 1], BF16)
            nc.vector.memset(proj1_ext[:, :, F1], 1.0)
            nc.sync.dma_start(
                proj1_ext[:, :, 0:F1],
                d_cout.ap().rearrange("(t p) f -> p t f", p=128)[:, :, 0:F1])
            s_tgt1_hi = cp.tile([128, NMC], BF16)
            nc.sync.dma_start(
                s_tgt1_hi,
                bass.AP(tensor=d_cout, offset=F1, ap=[[68, 128], [68 * 128, NMC]]))
            s_tgt1_lo = cp.tile([128, NMC], BF16)
            nc.sync.dma_start(
                s_tgt1_lo,
                bass.AP(tensor=d_cout, offset=F1 + 1, ap=[[68, 128], [68 * 128, NMC]]))
            s_tgt1_nat = cp.tile([128, NMC], F32)
            nc.vector.tensor_add(s_tgt1_nat, s_tgt1_hi, s_tgt1_lo)
            s1l0h = cp.tile([1, NL], F16)
            nc.vector.tensor_copy(s1l0h, s1l[0:1, :])
            nc.sync.dma_start(d_srow1[:, :], s1l0h)
            USrc1 = cp.tile([128, NL], F16)
            nc.sync.dma_start(USrc1, _bcast_rows(d_srow1, 0, NL))

            # ---------------- layer-1 main loop
            acc1 = psA.tile([F1 + 1, NL], F32, tag="acc0")
            for mcp in range(NMC // 4):
                tV = wp.tile([128, 2048], F32, tag="V", bufs=2)
                tP = wp.tile([128, 2048], BF16, tag="P", bufs=2)
                for sub in range(4):
                    mc = 4 * mcp + sub
                    nc.vector._custom_dve(
                        SCORE_LRELU,
                        out=tV[:, sub * 512:(sub + 1) * 512],
                        in0=USrc1,
                        in1=m_mask[:, mc, :],
                        s0=NEG,
                        s1=s_tgt1_nat[:, mc:mc + 1])
                nc.scalar.activation(tP, tV, AF.Exp)
                for sub in range(4):
                    mc = 4 * mcp + sub
                    nc.tensor.matmul(
                        acc1,
                        lhsT=proj1_ext[:, mc, :],
                        rhs=tP[:, sub * 512:(sub + 1) * 512],
                        start=(mc == 0), stop=(mc == NMC - 1),
                        skip_group_check=True)
            tOut = wp.tile([F1 + 1, NL], F32, tag="out", bufs=1)
            nc.scalar.copy(tOut, acc1)
            nc.sync.dma_start(d_out[:, :], tOut)

    nc.finalize()
    return nc


_CACHED = {}


def _get_program():
    if "nc" not in _CACHED:
        _CACHED["nc"] = build_program()
    return _CACHED["nc"]


def kernel(node_features, connectivity_mask, W0, b0, a_src0, a_tgt0,
           W1, b1, a_src1, a_tgt1):
    x = np.asarray(node_features, np.float32)
    mask = np.asarray(connectivity_mask, np.float32)
    W0 = np.asarray(W0, np.float32); b0 = np.asarray(b0, np.float32)
    W1 = np.asarray(W1, np.float32); b1 = np.asarray(b1, np.float32)
    a_src0 = np.asarray(a_src0, np.float32); a_tgt0 = np.asarray(a_tgt0, np.float32)
    a_src1 = np.asarray(a_src1, np.float32); a_tgt1 = np.asarray(a_tgt1, np.float32)

    maskT = np.ascontiguousarray(mask.T).astype(bf16)
    xT = np.ascontiguousarray(x.T)                       # [FIN, N]
    xT_hi, xT_lo = _hilo(xT)
    W0_hi, W0_lo = _hilo(W0)
    A0 = np.zeros((OUT0, 2 * H0), np.float32)
    for h in range(H0):
        A0[h * F0:(h + 1) * F0, h] = a_src0[0, h]
        A0[h * F0:(h + 1) * F0, H0 + h] = a_tgt0[0, h]
    WA0 = W0 @ A0
    WA0_hi, WA0_lo = _hilo(WA0)
    sb0 = (b0 @ A0).astype(np.float32)
    b0r_hi, b0r_lo = _hilo(b0.reshape(1, OUT0))
    W1_hi, W1_lo = _hilo(W1)
    W1s_hi = np.ascontiguousarray(W1_hi.reshape(H0, F0, F1).transpose(1, 0, 2))
    W1s_lo = np.ascontiguousarray(W1_lo.reshape(H0, F0, F1).transpose(1, 0, 2))
    A1 = np.zeros((F1, 2), np.float32)
    A1[:, 0] = a_src1[0, 0]
    A1[:, 1] = a_tgt1[0, 0]
    WA1 = W1 @ A1
    WA1_hi, WA1_lo = _hilo(WA1)
    WA1s_hi = np.ascontiguousarray(WA1_hi.reshape(H0, F0, 2).transpose(1, 0, 2))
    WA1s_lo = np.ascontiguousarray(WA1_lo.reshape(H0, F0, 2).transpose(1, 0, 2))
    sb1 = (b1 @ A1).reshape(2, 1).astype(np.float32)
    b1r_hi, b1r_lo = _hilo(b1.reshape(1, F1))

    shared = {
        "xT_hi": xT_hi, "xT_lo": xT_lo,
        "W0_hi": W0_hi, "W0_lo": W0_lo,
        "WA0_hi": WA0_hi, "WA0_lo": WA0_lo,
        "sb0": sb0.reshape(2 * H0, 1).copy(), "sb0r": sb0,
        "b0r_hi": b0r_hi, "b0r_lo": b0r_lo,
        "W1s_hi": W1s_hi, "W1s_lo": W1s_lo,
        "WA1s_hi": WA1s_hi, "WA1s_lo": WA1s_lo,
        "b1r_hi": b1r_hi, "b1r_lo": b1r_lo,
        "sb1": sb1,
    }
    in_maps = []
    for c in range(NCORES):
        cs = c * NL
        m = dict(shared)
        m["maskT"] = np.ascontiguousarray(maskT[:, cs:cs + NL])
        m["xTl_hi"] = np.ascontiguousarray(xT_hi[:, cs:cs + NL])
        m["xTl_lo"] = np.ascontiguousarray(xT_lo[:, cs:cs + NL])
        in_maps.append(m)

    nc = _get_program()
    trace = bool(int(os.environ.get("GAT_TRACE", "0")))
    res = run_bass_kernel_spmd(nc, in_maps, core_ids=list(range(NCORES)),
                               trace=trace)
    _CACHED["last_result"] = res

    out = np.empty((N, F1), np.float32)
    for c in range(NCORES):
        R = res.results[c]["outT"]
        out[c * NL:(c + 1) * NL, :] = (R[0:F1, :] / R[F1:F1 + 1, :]).T
    return out


# revision 21
# speedup vs baseline: 1.0112x; 1.0112x over previous
"""Bass/Trainium2 kernel for the 2-layer GAT problem (nn_GAT_79998060855611).

Strategy: row-shard the N x N attention across 8 NeuronCores (each core owns
N/8 = 512 query nodes).  Scores live in transposed layout [m, n] so the
softmax denominator falls out of the value-aggregation matmul via a ones
column in its lhsT.  The whole score computation
    V[m, n] = lrelu(s_src[n] + s_tgt[m] + mask[n, m])
is ONE custom DVE op per tile: Src0 = s_src broadcast rows (f32), Src1 =
transposed mask (bf16), C1 = per-partition s_tgt column (f32).  exp is a
single ScalarE pass.  All matmuls are bf16 with hi/lo-split accumulation.
The layers exchange the tiny layer-1 projections via an on-chip AllGather.
The final normalisation (divide by the attention row-sums) and the output
transpose are done on the host ([65, 512] per core).
"""
import os
import numpy as np
import ml_dtypes

import concourse.bass as bass
import concourse.tile as tile
import concourse.dve_ops as dve_ops
from concourse import bacc, mybir
from concourse.bass_utils import run_bass_kernel_spmd
from concourse.dve_spec import Spec, Src0, Src1, C0, C1, maxx, lower
from concourse.dve_uop import DveOpSpec

bf16 = ml_dtypes.bfloat16
F32 = mybir.dt.float32
BF16 = mybir.dt.bfloat16
F16 = mybir.dt.float16
AF = mybir.ActivationFunctionType

N = 4096
FIN = 128
H0, F0 = 8, 64
OUT0 = H0 * F0          # 512
F1 = 64
NCORES = 8
NL = N // NCORES        # 512 queries per core
NEG = 0.2
NMC = N // 128          # 32 m-chunks


# ---------------------------------------------------------------- custom op
def _register_score_lrelu():
    """out = lrelu(Src0 + Src1 + C1) with slope C0."""
    name = "GAT_SCORE_LRELU"
    for op in dve_ops.OPS:
        if op.name == name:
            return op
    u = Src0 + Src1 + C1
    spec = Spec(body=maxx(u * C0, u))
    opcode = dve_ops._CUSTOM_DVE_ROW_BASE + len(dve_ops.OPS)
    shas = {}
    for ver in ("v3", "v4"):
        s = DveOpSpec(name=name, opcode=opcode, uops=lower(spec, ver=ver), rd1_en=True)
        shas[ver] = s.sha(ver)
    op = dve_ops.DveOp(name, spec, subdim=False, uops_sha=shas)
    dve_ops.OPS.append(op)
    dve_ops._SUB_OPCODE_FOR_NAME[name] = opcode
    dve_ops.CUSTOM_DVE_SPECS[name] = spec
    return op


SCORE_LRELU = _register_score_lrelu()


def _hilo(a):
    hi = a.astype(bf16)
    lo = (a - hi.astype(np.float32)).astype(bf16)
    return hi, lo


def _bcast_rows(d_handle, row, width, parts=128):
    """DRAM AP reading one row of a [rows, width] f32 tensor, replicated
    across `parts` partitions."""
    return bass.AP(tensor=d_handle, offset=row * width,
                   ap=[[0, parts], [1, width]])


# ---------------------------------------------------------------- program
def build_program(sim_mode=False):
    nc = bacc.Bacc("TRN2", target_bir_lowering=False, debug=False,
                   num_devices=NCORES)

    def din(name, shape, dt=BF16):
        return nc.dram_tensor(name, shape, dt, kind="ExternalInput")

    d_xT_hi = din("xT_hi", [FIN, N]); d_xT_lo = din("xT_lo", [FIN, N])
    d_xTl_hi = din("xTl_hi", [FIN, NL]); d_xTl_lo = din("xTl_lo", [FIN, NL])
    d_W0_hi = din("W0_hi", [FIN, OUT0]); d_W0_lo = din("W0_lo", [FIN, OUT0])
    d_WA0_hi = din("WA0_hi", [FIN, 2 * H0]); d_WA0_lo = din("WA0_lo", [FIN, 2 * H0])
    d_sb0 = din("sb0", [2 * H0, 1], F32)        # b0 @ A0, column
    d_sb0r = din("sb0r", [2 * H0], F32)         # b0 @ A0, row (broadcast source)
    d_b0r_hi = din("b0r_hi", [1, OUT0]); d_b0r_lo = din("b0r_lo", [1, OUT0])
    d_W1s_hi = din("W1s_hi", [F0, H0, F1]); d_W1s_lo = din("W1s_lo", [F0, H0, F1])
    d_WA1s_hi = din("WA1s_hi", [F0, H0, 2]); d_WA1s_lo = din("WA1s_lo", [F0, H0, 2])
    d_b1r_hi = din("b1r_hi", [1, F1]); d_b1r_lo = din("b1r_lo", [1, F1])
    d_sb1 = din("sb1", [2, 1], F32)
    d_mask = din("maskT", [N, NL])

    d_out = nc.dram_tensor("outT", [F1 + 1, NL], F32, kind="ExternalOutput")

    # internal DRAM: collective bounce + row-broadcast scratch
    d_cin = nc.dram_tensor("gat_cin", [NL * 66], BF16)
    d_cout = nc.dram_tensor("gat_cout", [NCORES * NL * 66], BF16, addr_space="Shared")
    d_srows = nc.dram_tensor("gat_srows", [2 * H0, NL], F16)
    d_srow1 = nc.dram_tensor("gat_srow1", [1, NL], F16)

    with tile.TileContext(nc) as tc:
        with (
            tc.tile_pool(name="const", bufs=1) as cp,
            tc.tile_pool(name="work", bufs=3) as wp,
            tc.tile_pool(name="psS", bufs=2, space="PSUM") as psS,
            tc.tile_pool(name="psAcc", bufs=1, space="PSUM") as psA,
        ):
            # ---------------- loads
            t_sb0 = cp.tile([2 * H0, 1], F32); nc.sync.dma_start(t_sb0, d_sb0[:, :])
            t_sb0b = cp.tile([128, 2 * H0], F32)
            nc.sync.dma_start(t_sb0b, bass.AP(tensor=d_sb0r, offset=0,
                                              ap=[[0, 128], [1, 2 * H0]]))
            t_xT_hi = cp.tile([FIN, N], BF16); nc.sync.dma_start(t_xT_hi, d_xT_hi[:, :])
            t_xT_lo = cp.tile([FIN, N], BF16); nc.sync.dma_start(t_xT_lo, d_xT_lo[:, :])
            t_xTl_hi = cp.tile([FIN, NL], BF16); nc.sync.dma_start(t_xTl_hi, d_xTl_hi[:, :])
            t_xTl_lo = cp.tile([FIN, NL], BF16); nc.sync.dma_start(t_xTl_lo, d_xTl_lo[:, :])
            t_W0_hi = cp.tile([FIN, OUT0], BF16); nc.sync.dma_start(t_W0_hi, d_W0_hi[:, :])
            t_W0_lo = cp.tile([FIN, OUT0], BF16); nc.sync.dma_start(t_W0_lo, d_W0_lo[:, :])
            t_WA0_hi = cp.tile([FIN, 2 * H0], BF16); nc.sync.dma_start(t_WA0_hi, d_WA0_hi[:, :])
            t_WA0_lo = cp.tile([FIN, 2 * H0], BF16); nc.sync.dma_start(t_WA0_lo, d_WA0_lo[:, :])
            t_b0r_hi = cp.tile([1, OUT0], BF16); nc.sync.dma_start(t_b0r_hi, d_b0r_hi[:, :])
            t_b0r_lo = cp.tile([1, OUT0], BF16); nc.sync.dma_start(t_b0r_lo, d_b0r_lo[:, :])
            t_W1s_hi = cp.tile([F0, H0, F1], BF16); nc.sync.dma_start(t_W1s_hi, d_W1s_hi[:, :, :])
            t_W1s_lo = cp.tile([F0, H0, F1], BF16); nc.sync.dma_start(t_W1s_lo, d_W1s_lo[:, :, :])
            t_WA1s_hi = cp.tile([F0, H0, 2], BF16); nc.sync.dma_start(t_WA1s_hi, d_WA1s_hi[:, :, :])
            t_WA1s_lo = cp.tile([F0, H0, 2], BF16); nc.sync.dma_start(t_WA1s_lo, d_WA1s_lo[:, :, :])
            t_b1r_hi = cp.tile([1, F1], BF16); nc.sync.dma_start(t_b1r_hi, d_b1r_hi[:, :])
            t_b1r_lo = cp.tile([1, F1], BF16); nc.sync.dma_start(t_b1r_lo, d_b1r_lo[:, :])
            t_sb1 = cp.tile([2, 1], F32); nc.sync.dma_start(t_sb1, d_sb1[:, :])

            m_mask = cp.tile([128, NMC, NL], BF16)
            _mview = d_mask.ap().rearrange("(t p) n -> p t n", p=128)
            for _mq in range(8):
                nc.scalar.dma_start(m_mask[:, _mq * 4:(_mq + 1) * 4, :],
                                    _mview[:, _mq * 4:(_mq + 1) * 4, :])

            t_ones1 = cp.tile([1, 128], BF16)
            nc.vector.memset(t_ones1, 1.0)
            t_ones164 = cp.tile([1, F0], BF16)
            nc.vector.memset(t_ones164, 1.0)

            # ---------------- s_src rows (local, T layout), broadcast via DRAM
            s_l0 = cp.tile([2 * H0, NL], F32)
            ps = psS.tile([128, 528], F32, tag="scratch")
            pp = ps[0:2 * H0, 0:NL]
            nc.tensor.matmul(pp, lhsT=t_WA0_hi, rhs=t_xTl_hi, start=True, stop=False)
            nc.tensor.matmul(pp, lhsT=t_WA0_lo, rhs=t_xTl_hi, start=False, stop=False)
            nc.tensor.matmul(pp, lhsT=t_WA0_hi, rhs=t_xTl_lo, start=False, stop=True)
            nc.scalar.activation(s_l0, pp, AF.Identity, bias=t_sb0)
            s_l0h = cp.tile([2 * H0, NL], F16)
            nc.vector.tensor_copy(s_l0h, s_l0)
            nc.sync.dma_start(d_srows[:, :], s_l0h)
            USrc = cp.tile([128, H0, NL], F16)
            for h in range(H0):
                nc.sync.dma_start(USrc[:, h, :], _bcast_rows(d_srows, h, NL))

            # ---------------- proj0_ext + s_all0_nat (s_tgt columns)
            proj0_ext = cp.tile([128, NMC, H0 * (F0 + 1)], BF16)
            p0v = proj0_ext.rearrange("p t (h f) -> p t h f", h=H0)
            nc.vector.memset(p0v[:, :, :, F0], 1.0)
            s_all0_nat = cp.tile([128, NMC, 2 * H0], F32)
            for mc in range(NMC):
                ps = psS.tile([128, 528], F32, tag="scratch")
                pp = ps[:, 0:OUT0]
                pq = ps[:, OUT0:OUT0 + 2 * H0]
                xs_hi = t_xT_hi[:, mc * 128:(mc + 1) * 128]
                xs_lo = t_xT_lo[:, mc * 128:(mc + 1) * 128]
                nc.tensor.matmul(pp, lhsT=xs_hi, rhs=t_W0_hi, start=True, stop=False)
                nc.tensor.matmul(pp, lhsT=xs_hi, rhs=t_W0_lo, start=False, stop=False)
                nc.tensor.matmul(pp, lhsT=xs_lo, rhs=t_W0_hi, start=False, stop=False)
                nc.tensor.matmul(pp, lhsT=t_ones1, rhs=t_b0r_hi, start=False, stop=False)
                nc.tensor.matmul(pp, lhsT=t_ones1, rhs=t_b0r_lo, start=False, stop=True)
                nc.tensor.matmul(pq, lhsT=xs_hi, rhs=t_WA0_hi, start=True, stop=False)
                nc.tensor.matmul(pq, lhsT=xs_hi, rhs=t_WA0_lo, start=False, stop=False)
                nc.tensor.matmul(pq, lhsT=xs_lo, rhs=t_WA0_hi, start=False, stop=True)
                nc.scalar.copy(p0v[:, mc, :, 0:F0],
                               pp.rearrange("p (h f) -> p h f", h=H0))
                nc.vector.tensor_add(s_all0_nat[:, mc, :], pq, t_sb0b)

            # ---------------- layer-0 main loop
            hT_hi = [cp.tile([F0, NL], BF16, name=f"hthi{h}", tag=f"hthi{h}")
                     for h in range(H0)]
            hT_lo = [cp.tile([F0, NL], BF16, name=f"htlo{h}", tag=f"htlo{h}")
                     for h in range(H0)]

            for g in range(2):
                accs = [psA.tile([F0 + 1, NL], F32, name=f"acc{hh}", tag=f"acc{hh}")
                        for hh in range(4)]
                for hh in range(4):
                    for mcp in range(NMC // 4):
                        h = 4 * g + hh
                        tV = wp.tile([128, 2048], F32, tag="V", bufs=2)
                        tP = wp.tile([128, 2048], BF16, tag="P", bufs=2)
                        for sub in range(4):
                            mc = 4 * mcp + sub
                            nc.vector._custom_dve(
                                SCORE_LRELU,
                                out=tV[:, sub * 512:(sub + 1) * 512],
                                in0=USrc[:, h, :],
                                in1=m_mask[:, mc, :],
                                s0=NEG,
                                s1=s_all0_nat[:, mc, H0 + h:H0 + h + 1])
                        nc.scalar.activation(tP, tV, AF.Exp)
                        for sub in range(4):
                            mc = 4 * mcp + sub
                            nc.tensor.matmul(
                                accs[hh],
                                lhsT=p0v[:, mc, h, :],
                                rhs=tP[:, sub * 512:(sub + 1) * 512],
                                start=(mc == 0), stop=(mc == NMC - 1),
                                skip_group_check=True)
                        # per-head normalize + split h_T (overlaps later heads)
                        if mcp == NMC // 4 - 1:
                            sums_stage = wp.tile([F0 + 1, NL], F32,
                                                 tag="sums_stage", bufs=2)
                            nc.scalar.copy(sums_stage[F0:F0 + 1, :],
                                           accs[hh][F0:F0 + 1, :])
                            sums_h = wp.tile([1, NL], F32, tag="sums0", bufs=2)
                            nc.sync.dma_start(sums_h[0:1, :],
                                              sums_stage[F0:F0 + 1, :])
                            rec = wp.tile([1, NL], F32, tag="rec", bufs=2)
                            nc.vector.reciprocal(rec, sums_h)
                            rec_hi = wp.tile([1, NL], BF16, tag="rechi", bufs=2)
                            nc.vector.tensor_copy(rec_hi, rec)
                            rec_hif = wp.tile([1, NL], F32, tag="rechif", bufs=2)
                            nc.vector.tensor_copy(rec_hif, rec_hi)
                            rec_lo = wp.tile([1, NL], BF16, tag="reclo", bufs=2)
                            nc.vector.tensor_sub(rec_lo, rec, rec_hif)
                            psB = psS.tile([128, 528], F32, tag="scratch")
                            pb = psB[0:F0, 0:NL]
                            nc.tensor.matmul(pb, lhsT=t_ones164, rhs=rec_hi,
                                             start=True, stop=False)
                            nc.tensor.matmul(pb, lhsT=t_ones164, rhs=rec_lo,
                                             start=False, stop=True)
                            tb = wp.tile([F0, NL], F32, tag="tb", bufs=2)
                            nc.scalar.copy(tb, pb)
                            tHf = wp.tile([F0, NL], F32, tag="tHf", bufs=2)
                            nc.vector.tensor_mul(tHf, accs[hh][0:F0, :], tb)
                            nc.vector.tensor_copy(hT_hi[h], tHf)
                            tmp = wp.tile([F0, NL], F32, tag="tmp", bufs=2)
                            nc.vector.tensor_copy(tmp, hT_hi[h])
                            nc.vector.tensor_sub(hT_lo[h], tHf, tmp)

            # ---------------- proj1 local [NL, F1] f32 + s_all1_T local [2, NL]
            proj1n = cp.tile([128, 4, F1], BF16)
            for nc4 in range(4):
                ps = psS.tile([128, 528], F32, tag="scratch")
                pp = ps[:, 0:F1]
                for h in range(H0):
                    w_hi = t_W1s_hi[:, h, :]
                    w_lo = t_W1s_lo[:, h, :]
                    hh_ = hT_hi[h][:, nc4 * 128:(nc4 + 1) * 128]
                    hl_ = hT_lo[h][:, nc4 * 128:(nc4 + 1) * 128]
                    nc.tensor.matmul(pp, lhsT=hh_, rhs=w_hi, start=(h == 0), stop=False)
                    nc.tensor.matmul(pp, lhsT=hh_, rhs=w_lo, start=False, stop=False)
                    nc.tensor.matmul(pp, lhsT=hl_, rhs=w_hi, start=False, stop=False)
                nc.tensor.matmul(pp, lhsT=t_ones1, rhs=t_b1r_hi, start=False, stop=False)
                nc.tensor.matmul(pp, lhsT=t_ones1, rhs=t_b1r_lo, start=False, stop=True)
                nc.scalar.copy(proj1n[:, nc4, :], pp)
            ps1 = psS.tile([128, 528], F32, tag="scratch")
            pp1 = ps1[0:2, 0:NL]
            for h in range(H0):
                wa_hi = t_WA1s_hi[:, h, :]
                wa_lo = t_WA1s_lo[:, h, :]
                nc.tensor.matmul(pp1, lhsT=wa_hi, rhs=hT_hi[h], start=(h == 0), stop=False)
                nc.tensor.matmul(pp1, lhsT=wa_lo, rhs=hT_hi[h], start=False, stop=False)
                nc.tensor.matmul(pp1, lhsT=wa_hi, rhs=hT_lo[h], start=False,
                                 stop=(h == H0 - 1))
            s1l = cp.tile([2, NL], F32)
            nc.scalar.activation(s1l, pp1, AF.Identity, bias=t_sb1)
            s1l_hi = cp.tile([2, NL], BF16)
            nc.vector.tensor_copy(s1l_hi, s1l)
            s1l_hif = cp.tile([2, NL], F32)
            nc.vector.tensor_copy(s1l_hif, s1l_hi)
            s1l_lo = cp.tile([2, NL], BF16)
            nc.vector.tensor_sub(s1l_lo, s1l, s1l_hif)

            # ---------------- gather: flat [proj1 NLx64 | tgt_hi NL | tgt_lo NL]
            nc.sync.dma_start(
                bass.AP(tensor=d_cin, offset=0, ap=[[F1, 128], [128 * F1, 4], [1, F1]]),
                proj1n)
            nc.sync.dma_start(
                bass.AP(tensor=d_cin, offset=NL * F1, ap=[[0, 1], [1, NL]]),
                s1l_hi[1:2, :])
            nc.sync.dma_start(
                bass.AP(tensor=d_cin, offset=NL * F1 + NL, ap=[[0, 1], [1, NL]]),
                s1l_lo[1:2, :])
            if sim_mode:
                for c in range(NCORES):
                    nc.sync.dma_start(d_cout[c * NL * 66:(c + 1) * NL * 66], d_cin[:])
            else:
                nc.gpsimd.collective_compute(
                    "AllGather", mybir.AluOpType.bypass,
                    replica_groups=[list(range(NCORES))],
                    ins=[d_cin.ap().opt()], outs=[d_cout.ap().opt()])

            # proj1_ext [128, 32, 65] bf16 + s_tgt1_nat [128, 32] f32
            proj1_ext = cp.tile([128, NMC, F1 + 1], BF16)
            nc.vector.memset(proj1_ext[:, :, F1], 1.0)
            BLK = NL * 66
            for c in range(NCORES):
                nc.sync.dma_start(
                    proj1_ext[:, 4 * c:4 * (c + 1), 0:F1],
                    bass.AP(tensor=d_cout, offset=c * BLK,
                            ap=[[F1, 128], [128 * F1, 4], [1, F1]]))
            s_tgt1_hi = cp.tile([128, NMC], BF16)
            s_tgt1_lo = cp.tile([128, NMC], BF16)
            for c in range(NCORES):
                nc.sync.dma_start(
                    s_tgt1_hi[:, 4 * c:4 * (c + 1)],
                    bass.AP(tensor=d_cout, offset=c * BLK + NL * F1,
                            ap=[[1, 128], [128, 4]]))
                nc.sync.dma_start(
                    s_tgt1_lo[:, 4 * c:4 * (c + 1)],
                    bass.AP(tensor=d_cout, offset=c * BLK + NL * F1 + NL,
                            ap=[[1, 128], [128, 4]]))
            s_tgt1_nat = cp.tile([128, NMC], F32)
            nc.vector.tensor_add(s_tgt1_nat, s_tgt1_hi, s_tgt1_lo)
            s1l0h = cp.tile([1, NL], F16)
            nc.vector.tensor_copy(s1l0h, s1l[0:1, :])
            nc.sync.dma_start(d_srow1[:, :], s1l0h)
            USrc1 = cp.tile([128, NL], F16)
            nc.sync.dma_start(USrc1, _bcast_rows(d_srow1, 0, NL))

            # ---------------- layer-1 main loop
            acc1 = psA.tile([F1 + 1, NL], F32, tag="acc0")
            for mcp in range(NMC // 4):
                tV = wp.tile([128, 2048], F32, tag="V", bufs=2)
                tP = wp.tile([128, 2048], BF16, tag="P", bufs=2)
                for sub in range(4):
                    mc = 4 * mcp + sub
                    nc.vector._custom_dve(
                        SCORE_LRELU,
                        out=tV[:, sub * 512:(sub + 1) * 512],
                        in0=USrc1,
                        in1=m_mask[:, mc, :],
                        s0=NEG,
                        s1=s_tgt1_nat[:, mc:mc + 1])
                nc.scalar.activation(tP, tV, AF.Exp)
                for sub in range(4):
                    mc = 4 * mcp + sub
                    nc.tensor.matmul(
                        acc1,
                        lhsT=proj1_ext[:, mc, :],
                        rhs=tP[:, sub * 512:(sub + 1) * 512],
                        start=(mc == 0), stop=(mc == NMC - 1),
                        skip_group_check=True)
            tOut = wp.tile([F1 + 1, NL], F32, tag="out", bufs=1)
            nc.scalar.copy(tOut, acc1)
            nc.sync.dma_start(d_out[:, :], tOut)

    nc.finalize()
    return nc


_CACHED = {}


def _get_program():
    if "nc" not in _CACHED:
        _CACHED["nc"] = build_program()
    return _CACHED["nc"]


def kernel(node_features, connectivity_mask, W0, b0, a_src0, a_tgt0,
           W1, b1, a_src1, a_tgt1):
    x = np.asarray(node_features, np.float32)
    mask = np.asarray(connectivity_mask, np.float32)
    W0 = np.asarray(W0, np.float32); b0 = np.asarray(b0, np.float32)
    W1 = np.asarray(W1, np.float32); b1 = np.asarray(b1, np.float32)
    a_src0 = np.asarray(a_src0, np.float32); a_tgt0 = np.asarray(a_tgt0, np.float32)
    a_src1 = np.asarray(a_src1, np.float32); a_tgt1 = np.asarray(a_tgt1, np.float32)

    maskT = np.ascontiguousarray(mask.T).astype(bf16)
    xT = np.ascontiguousarray(x.T)                       # [FIN, N]
    xT_hi, xT_lo = _hilo(xT)
    W0_hi, W0_lo = _hilo(W0)
    A0 = np.zeros((OUT0, 2 * H0), np.float32)
    for h in range(H0):
        A0[h * F0:(h + 1) * F0, h] = a_src0[0, h]
        A0[h * F0:(h + 1) * F0, H0 + h] = a_tgt0[0, h]
    WA0 = W0 @ A0
    WA0_hi, WA0_lo = _hilo(WA0)
    sb0 = (b0 @ A0).astype(np.float32)
    b0r_hi, b0r_lo = _hilo(b0.reshape(1, OUT0))
    W1_hi, W1_lo = _hilo(W1)
    W1s_hi = np.ascontiguousarray(W1_hi.reshape(H0, F0, F1).transpose(1, 0, 2))
    W1s_lo = np.ascontiguousarray(W1_lo.reshape(H0, F0, F1).transpose(1, 0, 2))
    A1 = np.zeros((F1, 2), np.float32)
    A1[:, 0] = a_src1[0, 0]
    A1[:, 1] = a_tgt1[0, 0]
    WA1 = W1 @ A1
    WA1_hi, WA1_lo = _hilo(WA1)
    WA1s_hi = np.ascontiguousarray(WA1_hi.reshape(H0, F0, 2).transpose(1, 0, 2))
    WA1s_lo = np.ascontiguousarray(WA1_lo.reshape(H0, F0, 2).transpose(1, 0, 2))
    sb1 = (b1 @ A1).reshape(2, 1).astype(np.float32)
    b1r_hi, b1r_lo = _hilo(b1.reshape(1, F1))

    shared = {
        "xT_hi": xT_hi, "xT_lo": xT_lo,
        "W0_hi": W0_hi, "W0_lo": W0_lo,
        "WA0_hi": WA0_hi, "WA0_lo": WA0_lo,
        "sb0": sb0.reshape(2 * H0, 1).copy(), "sb0r": sb0,
        "b0r_hi": b0r_hi, "b0r_lo": b0r_lo,
        "W1s_hi": W1s_hi, "W1s_lo": W1s_lo,
        "WA1s_hi": WA1s_hi, "WA1s_lo": WA1s_lo,
        "b1r_hi": b1r_hi, "b1r_lo": b1r_lo,
        "sb1": sb1,
    }
    in_maps = []
    for c in range(NCORES):
        cs = c * NL
        m = dict(shared)
        m["maskT"] = np.ascontiguousarray(maskT[:, cs:cs + NL])
        m["xTl_hi"] = np.ascontiguousarray(xT_hi[:, cs:cs + NL])
        m["xTl_lo"] = np.ascontiguousarray(xT_lo[:, cs:cs + NL])
        in_maps.append(m)

    nc = _get_program()
    trace = bool(int(os.environ.get("GAT_TRACE", "0")))
    res = run_bass_kernel_spmd(nc, in_maps, core_ids=list(range(NCORES)),
                               trace=trace)
    _CACHED["last_result"] = res

    out = np.empty((N, F1), np.float32)
    for c in range(NCORES):
        R = res.results[c]["outT"]
        out[c * NL:(c + 1) * NL, :] = (R[0:F1, :] / R[F1:F1 + 1, :]).T
    return out
